# revision 51
# baseline (speedup 1.0000x reference)
"""Trainium2 Bass kernel: pre-LN multi-head attention (B=2, S=2048, d_model=1024, H=16).

Sharding: 8 cores = 2 batches x 4 head-groups. Core c handles batch c//4 and
heads 4*(c%4) .. 4*(c%4)+3 (a 256-wide slice of d_model).

Per-core device pipeline (all shapes per core):
  x_{q,k,v} [2048,1024]  --LN stats (DVE) + normalize (GPSIMD)-->  z  (token-major)
  z --PE transpose--> zT [1024,2048] (feature-major)
  QT/KT [256,2048] = W_slice @ zT   (feature-major, PE)
  V     [2048,256] token-major (zT as stationary), stored interleaved with a
        ones-column per head so the AV matmul also produces softmax denominators
  S^T   [k,q] tiles = K_h @ Q_h^T ; P^T = exp(S^T/8) (ACT, no max-subtraction:
        scores are ~N(0,1), exp is safe in fp32)
  ctx^T [64,q] = V_h^T @ P^T (fp32 PSUM accum; row 64 = sum_k P = denominator)
  y_partial [2048,1024] = ctx^T.T @ fo_slice^T  (PSUM -> DRAM)

Host: LayerNorm gamma/beta are folded into the projection weights/biases,
weights are pre-transposed to [in,out]; the 4 partial outputs per batch are
summed (row-parallel matmul gather-reduce) and fo_b added.
"""

import os
import numpy as np
import ml_dtypes
from contextlib import ExitStack

import concourse.bass as bass
import concourse.bacc as bacc
import concourse.tile as tile
from concourse import mybir
from concourse import bass_utils
from concourse.masks import make_identity

F32 = mybir.dt.float32
BF16 = mybir.dt.bfloat16

# All transcendentals in this kernel are Exp/Ln (rstd = exp(-0.5*ln(var+eps)),
# softmax exp, 1/denom = exp(-ln(d))). Exp and Ln coexist in the
# `natural_log_exp_and_others` ACT table set, but the table chooser picks
# per-function sets, emitting an ~2.7us ACT_TABLE_LOAD on every Exp<->Ln
# alternation. Strip Exp/Ln from every other set (names/indices preserved)
# so one resident set serves the whole kernel.
_orig_get_tables = bacc.get_activation_tables
_COMBINED = "natural_log_exp_and_others"


def _patched_get_tables(arch):
    tabs = _orig_get_tables(arch)
    if _COMBINED in tabs:
        drop = {mybir.ActivationFunctionType.Exp, mybir.ActivationFunctionType.Ln}
        tabs = {
            name: (fns if name == _COMBINED else fns - drop)
            for name, fns in tabs.items()
        }
    return tabs


bacc.get_activation_tables = _patched_get_tables

# ---- problem constants (hardcoded; kernel.py must be self-contained) ----
B, S, D = 2, 2048, 1024
NH_TOT, DH = 16, 64
N_CORES = 8
HPC = NH_TOT // 4          # 4 heads per core
HS = HPC * DH              # 256-wide feature slice per core
NT = S // 128              # 16 token tiles
NIC = D // 128             # 8 input-feature chunks
QB = 512                   # q-block width for attention
NQB = S // QB              # 4
LN_EPS = 1e-5
SCALE = 1.0 / np.sqrt(DH)  # 0.125

# dtype config (env-overridable for experiments)
_DT = {"f32": F32, "bf16": BF16}
XDT = _DT[os.environ.get("K_XDT", "bf16")]   # x input dtype (LN stats input)
TDT = _DT[os.environ.get("K_TDT", "bf16")]   # z / zT dtype (projection inputs)
WDT = _DT[os.environ.get("K_WDT", "bf16")]   # weight dtype
MDT = _DT[os.environ.get("K_MDT", "bf16")]   # attention matmul input dtype (QT/KT/V/P/CT)
TRANS_ENG = os.environ.get("K_TRANS", "pe")  # 'pe' or 'dma' (xbar) transposes

# schedule-tuning knobs (swept offline against TimelineSim)
TUNE = {
    "xp": 9, "zp": 8, "pb": 14, "st": 2, "mm": 2, "av": 2, "yp": 2, "ztp": 2,
    "copy_k": "act", "copy_q": "dve", "copy_v": "dve",
    "stats": "bn",       # 'bn' (bn_stats) or 'acc' (two accum_out passes)
    "v_first": True,     # interleave order: v-group before q-group
    "ablate": "none",    # model-only ablations: 'noln' | 'notrans'
    "lnmode": "fold",    # 'fold' (host xT + rank-1 LN fold) or 'transpose'
}

_NPDT = {F32: np.float32, BF16: ml_dtypes.bfloat16}


def build_nc():
    nc = bacc.Bacc("TRN2", target_bir_lowering=False, debug=False)

    xq = nc.dram_tensor("xq", [S, D], XDT, kind="ExternalInput")
    xk = nc.dram_tensor("xk", [S, D], XDT, kind="ExternalInput")
    xv = nc.dram_tensor("xv", [S, D], XDT, kind="ExternalInput")
    xqT = nc.dram_tensor("xqT", [D, S], TDT, kind="ExternalInput")
    xkT = nc.dram_tensor("xkT", [D, S], TDT, kind="ExternalInput")
    xvT = nc.dram_tensor("xvT", [D, S], TDT, kind="ExternalInput")
    a1q = nc.dram_tensor("a1q", [1, HS], WDT, kind="ExternalInput")
    a1k = nc.dram_tensor("a1k", [1, HS], WDT, kind="ExternalInput")
    a1v = nc.dram_tensor("a1v", [1, HS], WDT, kind="ExternalInput")
    wq = nc.dram_tensor("wq", [D, HS], WDT, kind="ExternalInput")
    wk = nc.dram_tensor("wk", [D, HS], WDT, kind="ExternalInput")
    wv = nc.dram_tensor("wv", [D, HS], WDT, kind="ExternalInput")
    fo = nc.dram_tensor("fo", [HS, D], WDT, kind="ExternalInput")
    bq = nc.dram_tensor("bq", [128, HS // 128], F32, kind="ExternalInput")
    bk = nc.dram_tensor("bk", [128, HS // 128], F32, kind="ExternalInput")
    bv = nc.dram_tensor("bv", [1, HS], F32, kind="ExternalInput")
    y = nc.dram_tensor("y", [S, D], F32, kind="ExternalOutput")

    with tile.TileContext(nc) as tc, ExitStack() as ctx:
        T = TUNE
        singles = ctx.enter_context(tc.tile_pool(name="singles", bufs=1))
        xp = ctx.enter_context(tc.tile_pool(name="xp", bufs=T["xp"]))
        zp = ctx.enter_context(tc.tile_pool(name="zp", bufs=T["zp"]))
        statp = ctx.enter_context(tc.tile_pool(name="statp", bufs=8))
        ztp = ctx.enter_context(tc.tile_pool(name="ztp", bufs=T["ztp"]))
        pp_mm = ctx.enter_context(
            tc.tile_pool(name="pp_mm", bufs=T["mm"], space="PSUM"))
        pp_st = ctx.enter_context(
            tc.tile_pool(name="pp_st", bufs=T["st"], space="PSUM"))
        pp_av = ctx.enter_context(
            tc.tile_pool(name="pp_av", bufs=T["av"], space="PSUM"))
        pb = ctx.enter_context(tc.tile_pool(name="pb", bufs=T["pb"]))
        recp = ctx.enter_context(tc.tile_pool(name="recp", bufs=2))
        yp = ctx.enter_context(tc.tile_pool(name="yp", bufs=T["yp"]))

        # --- constants ---
        identity = singles.tile([128, 128], TDT)
        make_identity(nc, identity[:, :])
        identity_f = singles.tile([128, 128], F32)
        make_identity(nc, identity_f[:, :])
        a1_sbs = {}
        for name, ad in (("q", a1q), ("k", a1k), ("v", a1v)):
            a1_sb = singles.tile([1, HS], WDT, tag=f"a1{name}")
            nc.sync.dma_start(out=a1_sb[:, :], in_=ad[:, :])
            a1_sbs[name] = a1_sb
        nrp = ctx.enter_context(tc.tile_pool(name="nrp", bufs=3))
        rbp = ctx.enter_context(tc.tile_pool(name="rbp", bufs=2))
        eps_t = singles.tile([128, 1], F32)
        nc.vector.memset(eps_t[:, :], LN_EPS)
        bq_sb = singles.tile([128, 2], F32)
        nc.sync.dma_start(out=bq_sb[:, :], in_=bq[:, :])
        bk_sb = singles.tile([128, 2], F32)
        nc.sync.dma_start(out=bk_sb[:, :], in_=bk[:, :])
        bv_sb = singles.tile([128, HS], F32)
        nc.sync.dma_start(out=bv_sb[:, :], in_=bv[:, :].to_broadcast([128, HS]))

        w_sbs = {}
        for name, wd in (("q", wq), ("k", wk), ("v", wv)):
            w_sb = singles.tile([128, NIC, HS], WDT, tag=f"w{name}")
            nc.sync.dma_start(
                out=w_sb[:, :, :],
                in_=wd[:, :].rearrange("(c p) n -> p c n", p=128))
            w_sbs[name] = w_sb
        fo_sb = singles.tile([128, 2, D], WDT)
        nc.sync.dma_start(
            out=fo_sb[:, :, :], in_=fo[:, :].rearrange("(c p) n -> p c n", p=128))

        # scratch for accum_out stat passes (WAW on the same engine is benign)
        trash_t = singles.tile([128, D], XDT, tag="trash")
        # feature-major Q^T / K^T [256, 2048] as [128, chunk, tok]
        QT = singles.tile([128, 2, S], MDT, tag="QT")
        KT = singles.tile([128, 2, S], MDT, tag="KT")
        # token-major V, heads interleaved with a ones column: [128, tok_tile, h, 65]
        V65 = singles.tile([128, NT, HPC, DH + 1], MDT, tag="V65")
        nc.vector.memset(V65[:, :, :, DH:DH + 1], 1.0)
        # feature-major context [256, 2048]
        CT = singles.tile([128, 2, S], MDT, tag="CT")

        def ln_fold_project(x_dram, xT_dram, wname, mode):
            """LN folded into the projection: raw x^T comes pre-transposed
            from the host; per-token stats are computed from token-major
            tiles; the -mu correction enters the projection PSUM as a rank-1
            (K=1) matmul and the rstd factor is applied as an output-column
            (q/k, via a gpsimd partition-broadcast) or output-row (v) scale."""
            xt_sb = ztp.tile([128, NIC, S], TDT, tag="zt")
            w_sb = w_sbs[wname]
            a1_sb = a1_sbs[wname]
            xT_r = xT_dram[:, :].rearrange("(c p) t -> p c t", p=128)

            def do_group(grp):
                gs = slice(grp * 512, (grp + 1) * 512)
                nc.sync.dma_start(out=xt_sb[:, :, gs], in_=xT_r[:, :, gs])
                rst4 = statp.tile([128, 4], F32, tag="rst4")
                nm4 = statp.tile([128, 4], F32, tag="nm4")
                if T["ablate"] == "nostat":
                    nc.vector.memset(rst4[:, :], 1.0)
                    nc.vector.memset(nm4[:, :], 0.0)
                for jj, j in enumerate(range(4 * grp, 4 * grp + 4)):
                    if T["ablate"] == "nostat":
                        break
                    x_t = xp.tile([128, D], XDT)
                    nc.sync.dma_start(out=x_t[:, :],
                                      in_=x_dram[j * 128:(j + 1) * 128, :])
                    st = statp.tile([128, 2, 6], F32, tag="st")
                    for g in range(2):
                        nc.vector.bn_stats(out=st[:, g, :],
                                           in_=x_t[:, g * 512:(g + 1) * 512])
                    mv = statp.tile([128, 2], F32, tag="mv")
                    nc.vector.bn_aggr(out=mv[:, :], in_=st[:, :, :])
                    lnv = statp.tile([128, 1], F32, tag="lnv")
                    nc.scalar.activation(lnv[:, :], mv[:, 1:2],
                                         mybir.ActivationFunctionType.Ln,
                                         bias=eps_t[:, :], scale=1.0)
                    nc.scalar.activation(rst4[:, jj:jj + 1], lnv[:, :],
                                         mybir.ActivationFunctionType.Exp,
                                         scale=-0.5)
                    nc.vector.tensor_scalar(
                        out=nm4[:, jj:jj + 1], in0=mv[:, 0:1], scalar1=-1.0,
                        scalar2=None, op0=mybir.AluOpType.mult)
                # transpose the per-token stat columns into free-major rows
                # ([128,1] -> [1,128] each, assembled on partition 0)
                tp_n = pp_mm.tile([1, 4, 128], F32, tag="mm")
                tp_r = pp_mm.tile([1, 4, 128], F32, tag="mm")
                for jj in range(4):
                    nc.tensor.transpose(tp_n[:, jj, :], nm4[:, jj:jj + 1],
                                        identity_f[:, :])
                    nc.tensor.transpose(tp_r[:, jj, :], rst4[:, jj:jj + 1],
                                        identity_f[:, :])
                nmT = nrp.tile([1, 512], TDT, tag="nmT")
                nc.vector.tensor_copy(nmT[:, :], tp_n[:, :, :])
                rT = nrp.tile([1, 512], F32, tag="rT")
                nc.vector.tensor_copy(rT[:, :], tp_r[:, :, :])

                if mode == "fm":
                    rbc = rbp.tile([128, 512], F32, tag="rbc")
                    nc.gpsimd.partition_broadcast(rbc[:, :], rT[:, :])
                    dst = QT if wname == "q" else KT
                    b_sb = bq_sb if wname == "q" else bk_sb
                    for m in range(2):
                        ps = pp_mm.tile([128, 512], F32, tag="mm")
                        for ic in range(NIC):
                            nc.tensor.matmul(
                                ps[:, :],
                                lhsT=w_sb[:, ic, m * 128:(m + 1) * 128],
                                rhs=xt_sb[:, ic, gs],
                                start=(ic == 0), stop=False)
                        nc.tensor.matmul(
                            ps[:, :], lhsT=a1_sb[:, m * 128:(m + 1) * 128],
                            rhs=nmT[:, :], start=False, stop=True)
                        nc.vector.tensor_tensor(
                            out=dst[:, m, gs], in0=ps[:, :], in1=rbc[:, :],
                            op=mybir.AluOpType.mult)
                        nc.vector.tensor_scalar(
                            out=dst[:, m, gs], in0=dst[:, m, gs],
                            scalar1=b_sb[:, m:m + 1], scalar2=None,
                            op0=mybir.AluOpType.add)
                else:
                    for jj, j in enumerate(range(4 * grp, 4 * grp + 4)):
                        ps = pp_mm.tile([128, HS], F32, tag="mm")
                        for ic in range(NIC):
                            nc.tensor.matmul(
                                ps[:, :],
                                lhsT=xt_sb[:, ic, j * 128:(j + 1) * 128],
                                rhs=w_sb[:, ic, :],
                                start=(ic == 0), stop=False)
                        nc.tensor.matmul(
                            ps[:, :], lhsT=nmT[:, jj * 128:(jj + 1) * 128],
                            rhs=a1_sb[:, :], start=False, stop=True)
                        vs = zp.tile([128, HS], MDT, tag="vs")
                        nc.vector.tensor_scalar(
                            out=vs[:, :], in0=ps[:, :],
                            scalar1=rst4[:, jj:jj + 1], scalar2=None,
                            op0=mybir.AluOpType.mult)
                        for h in range(HPC):
                            nc.vector.tensor_tensor(
                                out=V65[:, j, h, 0:DH],
                                in0=vs[:, h * DH:(h + 1) * DH],
                                in1=bv_sb[:, h * DH:(h + 1) * DH],
                                op=mybir.AluOpType.add)

            return do_group

        def ln_transpose_project(x_dram, wname, mode, copy_eng):
            """Returns do_group(g): LN -> z -> zT -> projection for token tiles
            4g..4g+3. mode: 'fm' (feature-major out into QT/KT) or 'tm'
            (token-major out into V65). copy_eng: 'act' or 'dve' for the
            PSUM->SBUF transpose copyback."""
            zt = ztp.tile([128, NIC, S], TDT, tag="zt")
            w_sb = w_sbs[wname]

            def proj_group(n):
                # token-range n*512:(n+1)*512 of zT is complete
                if mode == "fm":
                    dst = QT if wname == "q" else KT
                    b_sb = bq_sb if wname == "q" else bk_sb
                    for m in range(2):
                        ps = pp_mm.tile([128, 512], F32, tag="mm")
                        for ic in range(NIC):
                            nc.tensor.matmul(
                                ps[:, :],
                                lhsT=w_sb[:, ic, m * 128:(m + 1) * 128],
                                rhs=zt[:, ic, n * 512:(n + 1) * 512],
                                start=(ic == 0), stop=(ic == NIC - 1))
                        nc.vector.tensor_scalar(
                            out=dst[:, m, n * 512:(n + 1) * 512], in0=ps[:, :],
                            scalar1=b_sb[:, m:m + 1],
                            scalar2=None, op0=mybir.AluOpType.add)
                else:
                    for j in range(4 * n, 4 * n + 4):
                        ps = pp_mm.tile([128, HS], F32, tag="mm")
                        for ic in range(NIC):
                            nc.tensor.matmul(
                                ps[:, :],
                                lhsT=zt[:, ic, j * 128:(j + 1) * 128],
                                rhs=w_sb[:, ic, :],
                                start=(ic == 0), stop=(ic == NIC - 1))
                        for h in range(HPC):
                            nc.vector.tensor_tensor(
                                out=V65[:, j, h, 0:DH],
                                in0=ps[:, h * DH:(h + 1) * DH],
                                in1=bv_sb[:, h * DH:(h + 1) * DH],
                                op=mybir.AluOpType.add)

            def do_group(grp):
                for j in range(4 * grp, 4 * grp + 4):
                    x_t = xp.tile([128, D], XDT)
                    nc.sync.dma_start(out=x_t[:, :],
                                      in_=x_dram[j * 128:(j + 1) * 128, :])
                    if T["ablate"] in ("noln", "notrans"):
                        if T["ablate"] == "notrans":
                            nc.sync.dma_start(
                                out=zt[:, :, j * 128:(j + 1) * 128],
                                in_=x_dram[j * 128:(j + 1) * 128, :].rearrange(
                                    "p (c q) -> p c q", c=NIC))
                        else:
                            tp = pp_mm.tile([128, NIC, 128], TDT, tag="mm")
                            for ic in range(NIC):
                                nc.tensor.transpose(
                                    tp[:, ic, :], x_t[:, ic * 128:(ic + 1) * 128],
                                    identity[:, :])
                            nc.vector.tensor_copy(
                                zt[:, :, j * 128:(j + 1) * 128], tp[:, :, :])
                        continue
                    if T["stats"] == "bn":
                        st = statp.tile([128, 2, 6], F32, tag="st")
                        for g in range(2):
                            nc.vector.bn_stats(out=st[:, g, :],
                                               in_=x_t[:, g * 512:(g + 1) * 512])
                        mv = statp.tile([128, 2], F32, tag="mv")
                        nc.vector.bn_aggr(out=mv[:, :], in_=st[:, :, :])
                        mu_ap, var_ap, var_scale = mv[:, 0:1], mv[:, 1:2], 1.0
                        mu_scale = 1.0
                    else:
                        # sum and sumsq via accum_out side-outputs (cheaper on
                        # DVE than bn_stats: 4x/2x modes apply)
                        sums = statp.tile([128, 2], F32, tag="mv")
                        trash = trash_t
                        nc.vector.tensor_scalar(
                            out=trash[:, :], in0=x_t[:, :], scalar1=1.0,
                            scalar2=None, op0=mybir.AluOpType.mult,
                            accum_out=sums[:, 0:1])
                        nc.vector.tensor_tensor_reduce(
                            out=trash[:, :], in0=x_t[:, :], in1=x_t[:, :],
                            scale=1.0, scalar=0.0, op0=mybir.AluOpType.mult,
                            op1=mybir.AluOpType.add, accum_out=sums[:, 1:2])
                        # var = (sumsq - sum^2/D)/D ; mu = sum/D
                        t1 = statp.tile([128, 1], F32, tag="st")
                        nc.vector.tensor_tensor(
                            out=t1[:, :], in0=sums[:, 0:1], in1=sums[:, 0:1],
                            op=mybir.AluOpType.mult)
                        varD = statp.tile([128, 1], F32, tag="varD")
                        nc.vector.tensor_scalar(
                            out=varD[:, :], in0=t1[:, :], scalar1=-1.0 / D,
                            scalar2=sums[:, 1:2], op0=mybir.AluOpType.mult,
                            op1=mybir.AluOpType.add)
                        mu_ap, var_ap = sums[:, 0:1], varD[:, :]
                        var_scale, mu_scale = 1.0 / D, 1.0 / D
                    # rstd = exp(-0.5*ln(var+eps)) — Ln+Exp share one table set
                    lnv = statp.tile([128, 1], F32, tag="lnv")
                    nc.scalar.activation(lnv[:, :], var_ap,
                                         mybir.ActivationFunctionType.Ln,
                                         bias=eps_t[:, :], scale=var_scale)
                    rstd = statp.tile([128, 1], F32, tag="rstd")
                    nc.scalar.activation(rstd[:, :], lnv[:, :],
                                         mybir.ActivationFunctionType.Exp,
                                         scale=-0.5)
                    nmur = statp.tile([128, 1], F32, tag="nmur")
                    nc.vector.tensor_scalar(
                        out=nmur[:, :], in0=mu_ap, scalar1=rstd[:, :],
                        scalar2=-mu_scale, op0=mybir.AluOpType.mult,
                        op1=mybir.AluOpType.mult)
                    z = zp.tile([128, D], TDT)
                    nc.gpsimd.tensor_scalar(
                        out=z[:, :], in0=x_t[:, :], scalar1=rstd[:, :],
                        scalar2=nmur[:, :], op0=mybir.AluOpType.mult,
                        op1=mybir.AluOpType.add)
                    if TRANS_ENG == "dma":
                        # xbar transpose engine: SBUF->SBUF, no PE/ACT/DVE cost
                        for ic in range(NIC):
                            nc.sync.dma_start_transpose(
                                out=zt[:, ic, j * 128:(j + 1) * 128],
                                in_=z[:, ic * 128:(ic + 1) * 128])
                    else:
                        tp = pp_mm.tile([128, NIC, 128], TDT, tag="mm")
                        for ic in range(NIC):
                            nc.tensor.transpose(tp[:, ic, :],
                                                z[:, ic * 128:(ic + 1) * 128],
                                                identity[:, :])
                        if copy_eng == "act":
                            # Copy is present in every ACT table set (no reload)
                            nc.scalar.activation(zt[:, :, j * 128:(j + 1) * 128],
                                                 tp[:, :, :],
                                                 mybir.ActivationFunctionType.Copy)
                        else:
                            nc.vector.tensor_copy(zt[:, :, j * 128:(j + 1) * 128],
                                                  tp[:, :, :])
                proj_group(grp)

            return do_group

        # k first (full K needed by every S^T tile), then q and v interleaved
        # per 4-tile group: the attention exp stream starts as soon as QT's
        # first quarter exists, and AV(kt) streams behind V65[kt] production.
        if T["lnmode"] == "fold":
            sk = ln_fold_project(xk, xkT, "k", "fm")
            sq = ln_fold_project(xq, xqT, "q", "fm")
            sv = ln_fold_project(xv, xvT, "v", "tm")
        else:
            sk = ln_transpose_project(xk, "k", "fm", T["copy_k"])
            sq = ln_transpose_project(xq, "q", "fm", T["copy_q"])
            sv = ln_transpose_project(xv, "v", "tm", T["copy_v"])
        for g in range(4):
            sk(g)
        for g in range(4):
            if T["v_first"]:
                sv(g)
                sq(g)
            else:
                sq(g)
                sv(g)

        # --- attention (qb outer so the output projection can stream) ---
        QBW = T.get("qbw", QB)     # attention q-block width (512 or 1024)
        PAIR = 1024 // QBW         # k-tiles paired per PSUM tile/exp
        for qb in range(S // QBW):
            for h in range(HPC):
                hc, ho = h // 2, 64 * (h % 2)
                p_tiles = []
                for kt2 in range(NT // PAIR):
                    if T["ablate"] == "noexp":
                        p = pb.tile([128, PAIR, QBW], MDT)
                        nc.vector.memset(p[:, 0:1, 0:1], 0.0)
                        p_tiles.append(p)
                        continue
                    # PAIR k-tiles share one 2-bank PSUM tile so a single exp
                    # covers 1024 columns (amortizes ACT per-op overhead)
                    st_ps = pp_st.tile([128, PAIR, QBW], F32)
                    for i in range(PAIR):
                        kt = kt2 * PAIR + i
                        nc.tensor.matmul(
                            st_ps[:, i, :],
                            lhsT=KT[ho:ho + DH, hc, kt * 128:(kt + 1) * 128],
                            rhs=QT[ho:ho + DH, hc, qb * QBW:(qb + 1) * QBW],
                            start=True, stop=True)
                    p = pb.tile([128, PAIR, QBW], MDT)
                    nc.scalar.activation(p[:, :, :], st_ps[:, :, :],
                                         mybir.ActivationFunctionType.Exp,
                                         scale=float(SCALE))
                    p_tiles.append(p)
                if T["ablate"] == "noav":
                    continue
                av = pp_av.tile([DH + 1, QBW], F32)
                for kt in range(NT):
                    nc.tensor.matmul(
                        av[:, :],
                        lhsT=V65[:, kt, h, :],
                        rhs=p_tiles[kt // PAIR][:, kt % PAIR, :],
                        start=(kt == 0), stop=(kt == NT - 1))
                # 1/denom = exp(-ln(denom)) on ACT: avoids the 1-lane DVE
                # iterative divide (~3.3us per row) and stays in the one
                # resident Exp/Ln table set.
                lnd = recp.tile([1, QBW], F32, tag="lnd")
                nc.scalar.activation(lnd[:, :], av[DH:DH + 1, :],
                                     mybir.ActivationFunctionType.Ln)
                rec = recp.tile([1, QBW], F32, tag="rec")
                nc.scalar.activation(rec[:, :], lnd[:, :],
                                     mybir.ActivationFunctionType.Exp,
                                     scale=-1.0)
                recb = recp.tile([DH, QBW], F32, tag="recb")
                nc.gpsimd.partition_broadcast(recb[:, :], rec[:, :])
                nc.vector.tensor_tensor(
                    out=CT[ho:ho + DH, hc, qb * QBW:(qb + 1) * QBW],
                    in0=av[0:DH, :], in1=recb[:, :],
                    op=mybir.AluOpType.mult)

            # output projection for this qb's token tiles (all heads done)
            for j in range(qb * QBW // 128, (qb + 1) * QBW // 128):
                ys = yp.tile([128, D], F32)
                for n in range(2):
                    ps = pp_mm.tile([128, 512], F32, tag="mm")
                    for cc in range(2):
                        nc.tensor.matmul(
                            ps[:, :],
                            lhsT=CT[:, cc, j * 128:(j + 1) * 128],
                            rhs=fo_sb[:, cc, n * 512:(n + 1) * 512],
                            start=(cc == 0), stop=(cc == 1))
                    nc.vector.tensor_copy(ys[:, n * 512:(n + 1) * 512], ps[:, :])
                nc.sync.dma_start(out=y[j * 128:(j + 1) * 128, :], in_=ys[:, :])

    nc.compile()
    return nc


_NC_CACHE = {}


def _get_nc():
    key = (XDT, TDT, WDT, MDT)
    if key not in _NC_CACHE:
        _NC_CACHE[key] = build_nc()
    return _NC_CACHE[key]


def make_in_maps(q, k, v, ln_g, ln_b, wq_w, wq_b, wk_w, wk_b, wv_w, wv_b, fo_w, fo_b):
    """Host-side shard prep. Folds ln_g/ln_b into projection weights/biases."""
    xnp = _NPDT[XDT]
    wnp = _NPDT[WDT]
    g64 = ln_g.astype(np.float64)
    b64 = ln_b.astype(np.float64)
    in_maps = []
    for c in range(N_CORES):
        b = c // 4
        sl = slice((c % 4) * HS, (c % 4 + 1) * HS)
        tnp = _NPDT[TDT]
        m = {
            "xq": np.ascontiguousarray(q[b]).astype(xnp),
            "xk": np.ascontiguousarray(k[b]).astype(xnp),
            "xv": np.ascontiguousarray(v[b]).astype(xnp),
            "xqT": np.ascontiguousarray(q[b].T).astype(tnp),
            "xkT": np.ascontiguousarray(k[b].T).astype(tnp),
            "xvT": np.ascontiguousarray(v[b].T).astype(tnp),
        }
        for nm, w, bias in (("q", wq_w, wq_b), ("k", wk_w, wk_b), ("v", wv_w, wv_b)):
            ws = w[sl].astype(np.float64)          # [256, 1024]
            wg = ws * g64[None, :]                 # fold gamma
            cb = (ws @ b64 + bias[sl].astype(np.float64)).astype(np.float32)
            m["w" + nm] = np.ascontiguousarray(wg.T).astype(wnp)  # [1024, 256]
            m["a1" + nm] = wg.sum(axis=1).astype(np.float32).reshape(1, HS).astype(wnp)
            if nm == "v":
                m["bv"] = cb.reshape(1, HS)
            else:
                m["b" + nm] = np.ascontiguousarray(cb.reshape(2, 128).T)  # [128, 2]
        m["fo"] = np.ascontiguousarray(fo_w[:, sl].T).astype(wnp)  # [256, 1024]
        in_maps.append(m)
    return in_maps


def run_on_device(in_maps, trace=False):
    nc = _get_nc()
    return bass_utils.run_bass_kernel_spmd(
        nc, in_maps, core_ids=list(range(N_CORES)), trace=trace)


def assemble(res, fo_b):
    """Gather-reduce the row-parallel partials and add the output bias."""
    fo_b64 = np.asarray(fo_b, np.float64)
    out = np.empty((B, S, D), np.float32)
    for b in range(B):
        acc = np.zeros((S, D), np.float64)
        for c in range(b * 4, b * 4 + 4):
            acc += res.results[c]["y"].astype(np.float64)
        out[b] = (acc + fo_b64[None, :]).astype(np.float32)
    return out


def kernel(q, k, v, ln_g, ln_b, wq_w, wq_b, wk_w, wk_b, wv_w, wv_b, fo_w, fo_b):
    q = np.asarray(q, np.float32)
    k = np.asarray(k, np.float32)
    v = np.asarray(v, np.float32)
    in_maps = make_in_maps(q, k, v, np.asarray(ln_g, np.float32),
                           np.asarray(ln_b, np.float32),
                           np.asarray(wq_w, np.float32), np.asarray(wq_b, np.float32),
                           np.asarray(wk_w, np.float32), np.asarray(wk_b, np.float32),
                           np.asarray(wv_w, np.float32), np.asarray(wv_b, np.float32),
                           np.asarray(fo_w, np.float32), np.asarray(fo_b, np.float32))
    res = run_on_device(in_maps)
    return assemble(res, fo_b)


# revision 53
# speedup vs baseline: 1.1078x; 1.1078x over previous
"""Trainium2 Bass kernel: pre-LN multi-head attention (B=2, S=2048, d_model=1024, H=16).

Sharding: 8 cores = 2 batches x 4 head-groups. Core c handles batch c//4 and
heads 4*(c%4) .. 4*(c%4)+3 (a 256-wide slice of d_model).

Per-core device pipeline (all shapes per core):
  x_{q,k,v} [2048,1024]  --LN stats (DVE) + normalize (GPSIMD)-->  z  (token-major)
  z --PE transpose--> zT [1024,2048] (feature-major)
  QT/KT [256,2048] = W_slice @ zT   (feature-major, PE)
  V     [2048,256] token-major (zT as stationary), stored interleaved with a
        ones-column per head so the AV matmul also produces softmax denominators
  S^T   [k,q] tiles = K_h @ Q_h^T ; P^T = exp(S^T/8) (ACT, no max-subtraction:
        scores are ~N(0,1), exp is safe in fp32)
  ctx^T [64,q] = V_h^T @ P^T (fp32 PSUM accum; row 64 = sum_k P = denominator)
  y_partial [2048,1024] = ctx^T.T @ fo_slice^T  (PSUM -> DRAM)

Host: LayerNorm gamma/beta are folded into the projection weights/biases,
weights are pre-transposed to [in,out]; the 4 partial outputs per batch are
summed (row-parallel matmul gather-reduce) and fo_b added.
"""

import os
import numpy as np
import ml_dtypes
from contextlib import ExitStack

import concourse.bass as bass
import concourse.bacc as bacc
import concourse.tile as tile
from concourse import mybir
from concourse import bass_utils
from concourse.masks import make_identity

F32 = mybir.dt.float32
BF16 = mybir.dt.bfloat16

# All transcendentals in this kernel are Exp/Ln (rstd = exp(-0.5*ln(var+eps)),
# softmax exp, 1/denom = exp(-ln(d))). Exp and Ln coexist in the
# `natural_log_exp_and_others` ACT table set, but the table chooser picks
# per-function sets, emitting an ~2.7us ACT_TABLE_LOAD on every Exp<->Ln
# alternation. Strip Exp/Ln from every other set (names/indices preserved)
# so one resident set serves the whole kernel.
_orig_get_tables = bacc.get_activation_tables
_COMBINED = "natural_log_exp_and_others"


def _patched_get_tables(arch):
    tabs = _orig_get_tables(arch)
    if _COMBINED in tabs:
        drop = {mybir.ActivationFunctionType.Exp, mybir.ActivationFunctionType.Ln}
        tabs = {
            name: (fns if name == _COMBINED else fns - drop)
            for name, fns in tabs.items()
        }
    return tabs


bacc.get_activation_tables = _patched_get_tables

# ---- problem constants (hardcoded; kernel.py must be self-contained) ----
B, S, D = 2, 2048, 1024
NH_TOT, DH = 16, 64
N_CORES = 8
HPC = NH_TOT // 4          # 4 heads per core
HS = HPC * DH              # 256-wide feature slice per core
NT = S // 128              # 16 token tiles
NIC = D // 128             # 8 input-feature chunks
QB = 512                   # q-block width for attention
NQB = S // QB              # 4
LN_EPS = 1e-5
SCALE = 1.0 / np.sqrt(DH)  # 0.125

# dtype config (env-overridable for experiments)
_DT = {"f32": F32, "bf16": BF16}
XDT = _DT[os.environ.get("K_XDT", "bf16")]   # x input dtype (LN stats input)
TDT = _DT[os.environ.get("K_TDT", "bf16")]   # z / zT dtype (projection inputs)
WDT = _DT[os.environ.get("K_WDT", "bf16")]   # weight dtype
MDT = _DT[os.environ.get("K_MDT", "bf16")]   # attention matmul input dtype (QT/KT/V/P/CT)
TRANS_ENG = os.environ.get("K_TRANS", "pe")  # 'pe' or 'dma' (xbar) transposes

# schedule-tuning knobs (swept offline against TimelineSim)
TUNE = {
    "xp": 9, "zp": 8, "pb": 14, "st": 2, "mm": 2, "av": 2, "yp": 2, "ztp": 2,
    "copy_k": "act", "copy_q": "dve", "copy_v": "dve",
    "stats": "bn",       # 'bn' (bn_stats) or 'acc' (two accum_out passes)
    "v_first": True,     # interleave order: v-group before q-group
    "ablate": "none",    # model-only ablations: 'noln' | 'notrans'
    "lnmode": "transpose",  # 'transpose' or 'fold' (host xT + rank-1 LN fold)
    "qbw": 512,          # attention q-block width (512 or 1024)
}

_NPDT = {F32: np.float32, BF16: ml_dtypes.bfloat16}


def build_nc():
    nc = bacc.Bacc("TRN2", target_bir_lowering=False, debug=False)

    xq = nc.dram_tensor("xq", [S, D], XDT, kind="ExternalInput")
    xk = nc.dram_tensor("xk", [S, D], XDT, kind="ExternalInput")
    xv = nc.dram_tensor("xv", [S, D], XDT, kind="ExternalInput")
    xqT = nc.dram_tensor("xqT", [D, S], TDT, kind="ExternalInput")
    xkT = nc.dram_tensor("xkT", [D, S], TDT, kind="ExternalInput")
    xvT = nc.dram_tensor("xvT", [D, S], TDT, kind="ExternalInput")
    a1q = nc.dram_tensor("a1q", [1, HS], WDT, kind="ExternalInput")
    a1k = nc.dram_tensor("a1k", [1, HS], WDT, kind="ExternalInput")
    a1v = nc.dram_tensor("a1v", [1, HS], WDT, kind="ExternalInput")
    wq = nc.dram_tensor("wq", [D, HS], WDT, kind="ExternalInput")
    wk = nc.dram_tensor("wk", [D, HS], WDT, kind="ExternalInput")
    wv = nc.dram_tensor("wv", [D, HS], WDT, kind="ExternalInput")
    fo = nc.dram_tensor("fo", [HS, D], WDT, kind="ExternalInput")
    bq = nc.dram_tensor("bq", [128, HS // 128], F32, kind="ExternalInput")
    bk = nc.dram_tensor("bk", [128, HS // 128], F32, kind="ExternalInput")
    bv = nc.dram_tensor("bv", [1, HS], F32, kind="ExternalInput")
    y = nc.dram_tensor("y", [S, D], F32, kind="ExternalOutput")

    with tile.TileContext(nc) as tc, ExitStack() as ctx:
        T = TUNE
        singles = ctx.enter_context(tc.tile_pool(name="singles", bufs=1))
        xp = ctx.enter_context(tc.tile_pool(name="xp", bufs=T["xp"]))
        zp = ctx.enter_context(tc.tile_pool(name="zp", bufs=T["zp"]))
        statp = ctx.enter_context(tc.tile_pool(name="statp", bufs=8))
        ztp = ctx.enter_context(tc.tile_pool(name="ztp", bufs=T["ztp"]))
        pp_mm = ctx.enter_context(
            tc.tile_pool(name="pp_mm", bufs=T["mm"], space="PSUM"))
        pp_st = ctx.enter_context(
            tc.tile_pool(name="pp_st", bufs=T["st"], space="PSUM"))
        pp_av = ctx.enter_context(
            tc.tile_pool(name="pp_av", bufs=T["av"], space="PSUM"))
        pb = ctx.enter_context(tc.tile_pool(name="pb", bufs=T["pb"]))
        recp = ctx.enter_context(tc.tile_pool(name="recp", bufs=2))
        yp = ctx.enter_context(tc.tile_pool(name="yp", bufs=T["yp"]))

        # --- constants ---
        identity = singles.tile([128, 128], TDT)
        make_identity(nc, identity[:, :])
        identity_f = singles.tile([128, 128], F32)
        make_identity(nc, identity_f[:, :])
        a1_sbs = {}
        if T["lnmode"] == "fold":
            for name, ad in (("q", a1q), ("k", a1k), ("v", a1v)):
                a1_sb = singles.tile([1, HS], WDT, tag=f"a1{name}")
                nc.sync.dma_start(out=a1_sb[:, :], in_=ad[:, :])
                a1_sbs[name] = a1_sb
            nrp = ctx.enter_context(tc.tile_pool(name="nrp", bufs=3))
            rbp = ctx.enter_context(tc.tile_pool(name="rbp", bufs=2))
        eps_t = singles.tile([128, 1], F32)
        nc.vector.memset(eps_t[:, :], LN_EPS)
        bq_sb = singles.tile([128, 2], F32)
        nc.sync.dma_start(out=bq_sb[:, :], in_=bq[:, :])
        bk_sb = singles.tile([128, 2], F32)
        nc.sync.dma_start(out=bk_sb[:, :], in_=bk[:, :])
        bv_sb = singles.tile([128, HS], F32)
        nc.sync.dma_start(out=bv_sb[:, :], in_=bv[:, :].to_broadcast([128, HS]))

        w_sbs = {}
        for name, wd in (("q", wq), ("k", wk), ("v", wv)):
            w_sb = singles.tile([128, NIC, HS], WDT, tag=f"w{name}")
            nc.sync.dma_start(
                out=w_sb[:, :, :],
                in_=wd[:, :].rearrange("(c p) n -> p c n", p=128))
            w_sbs[name] = w_sb
        fo_sb = singles.tile([128, 2, D], WDT)
        nc.sync.dma_start(
            out=fo_sb[:, :, :], in_=fo[:, :].rearrange("(c p) n -> p c n", p=128))

        # scratch for accum_out stat passes (WAW on the same engine is benign)
        trash_t = singles.tile([128, D], XDT, tag="trash")
        # feature-major Q^T / K^T [256, 2048] as [128, chunk, tok]
        QT = singles.tile([128, 2, S], MDT, tag="QT")
        KT = singles.tile([128, 2, S], MDT, tag="KT")
        # token-major V, heads interleaved with a ones column: [128, tok_tile, h, 65]
        V65 = singles.tile([128, NT, HPC, DH + 1], MDT, tag="V65")
        nc.vector.memset(V65[:, :, :, DH:DH + 1], 1.0)
        # feature-major context [256, 2048]
        CT = singles.tile([128, 2, S], MDT, tag="CT")

        def ln_fold_project(x_dram, xT_dram, wname, mode):
            """LN folded into the projection: raw x^T comes pre-transposed
            from the host; per-token stats are computed from token-major
            tiles; the -mu correction enters the projection PSUM as a rank-1
            (K=1) matmul and the rstd factor is applied as an output-column
            (q/k, via a gpsimd partition-broadcast) or output-row (v) scale."""
            xt_sb = ztp.tile([128, NIC, S], TDT, tag="zt")
            w_sb = w_sbs[wname]
            a1_sb = a1_sbs[wname]
            xT_r = xT_dram[:, :].rearrange("(c p) t -> p c t", p=128)

            def do_group(grp):
                gs = slice(grp * 512, (grp + 1) * 512)
                nc.sync.dma_start(out=xt_sb[:, :, gs], in_=xT_r[:, :, gs])
                rst4 = statp.tile([128, 4], F32, tag="rst4")
                nm4 = statp.tile([128, 4], F32, tag="nm4")
                if T["ablate"] == "nostat":
                    nc.vector.memset(rst4[:, :], 1.0)
                    nc.vector.memset(nm4[:, :], 0.0)
                for jj, j in enumerate(range(4 * grp, 4 * grp + 4)):
                    if T["ablate"] == "nostat":
                        break
                    x_t = xp.tile([128, D], XDT)
                    nc.sync.dma_start(out=x_t[:, :],
                                      in_=x_dram[j * 128:(j + 1) * 128, :])
                    st = statp.tile([128, 2, 6], F32, tag="st")
                    for g in range(2):
                        nc.vector.bn_stats(out=st[:, g, :],
                                           in_=x_t[:, g * 512:(g + 1) * 512])
                    mv = statp.tile([128, 2], F32, tag="mv")
                    nc.vector.bn_aggr(out=mv[:, :], in_=st[:, :, :])
                    lnv = statp.tile([128, 1], F32, tag="lnv")
                    nc.scalar.activation(lnv[:, :], mv[:, 1:2],
                                         mybir.ActivationFunctionType.Ln,
                                         bias=eps_t[:, :], scale=1.0)
                    nc.scalar.activation(rst4[:, jj:jj + 1], lnv[:, :],
                                         mybir.ActivationFunctionType.Exp,
                                         scale=-0.5)
                    nc.vector.tensor_scalar(
                        out=nm4[:, jj:jj + 1], in0=mv[:, 0:1], scalar1=-1.0,
                        scalar2=None, op0=mybir.AluOpType.mult)
                # transpose the per-token stat columns into free-major rows
                # ([128,1] -> [1,128] each, assembled on partition 0)
                tp_n = pp_mm.tile([1, 4, 128], F32, tag="mm")
                tp_r = pp_mm.tile([1, 4, 128], F32, tag="mm")
                for jj in range(4):
                    nc.tensor.transpose(tp_n[:, jj, :], nm4[:, jj:jj + 1],
                                        identity_f[:, :])
                    nc.tensor.transpose(tp_r[:, jj, :], rst4[:, jj:jj + 1],
                                        identity_f[:, :])
                nmT = nrp.tile([1, 512], TDT, tag="nmT")
                nc.vector.tensor_copy(nmT[:, :], tp_n[:, :, :])
                rT = nrp.tile([1, 512], F32, tag="rT")
                nc.vector.tensor_copy(rT[:, :], tp_r[:, :, :])

                if mode == "fm":
                    rbc = rbp.tile([128, 512], F32, tag="rbc")
                    nc.gpsimd.partition_broadcast(rbc[:, :], rT[:, :])
                    dst = QT if wname == "q" else KT
                    b_sb = bq_sb if wname == "q" else bk_sb
                    for m in range(2):
                        ps = pp_mm.tile([128, 512], F32, tag="mm")
                        for ic in range(NIC):
                            nc.tensor.matmul(
                                ps[:, :],
                                lhsT=w_sb[:, ic, m * 128:(m + 1) * 128],
                                rhs=xt_sb[:, ic, gs],
                                start=(ic == 0), stop=False)
                        nc.tensor.matmul(
                            ps[:, :], lhsT=a1_sb[:, m * 128:(m + 1) * 128],
                            rhs=nmT[:, :], start=False, stop=True)
                        nc.vector.tensor_tensor(
                            out=dst[:, m, gs], in0=ps[:, :], in1=rbc[:, :],
                            op=mybir.AluOpType.mult)
                        nc.vector.tensor_scalar(
                            out=dst[:, m, gs], in0=dst[:, m, gs],
                            scalar1=b_sb[:, m:m + 1], scalar2=None,
                            op0=mybir.AluOpType.add)
                else:
                    for jj, j in enumerate(range(4 * grp, 4 * grp + 4)):
                        ps = pp_mm.tile([128, HS], F32, tag="mm")
                        for ic in range(NIC):
                            nc.tensor.matmul(
                                ps[:, :],
                                lhsT=xt_sb[:, ic, j * 128:(j + 1) * 128],
                                rhs=w_sb[:, ic, :],
                                start=(ic == 0), stop=False)
                        nc.tensor.matmul(
                            ps[:, :], lhsT=nmT[:, jj * 128:(jj + 1) * 128],
                            rhs=a1_sb[:, :], start=False, stop=True)
                        vs = zp.tile([128, HS], MDT, tag="vs")
                        nc.vector.tensor_scalar(
                            out=vs[:, :], in0=ps[:, :],
                            scalar1=rst4[:, jj:jj + 1], scalar2=None,
                            op0=mybir.AluOpType.mult)
                        for h in range(HPC):
                            nc.vector.tensor_tensor(
                                out=V65[:, j, h, 0:DH],
                                in0=vs[:, h * DH:(h + 1) * DH],
                                in1=bv_sb[:, h * DH:(h + 1) * DH],
                                op=mybir.AluOpType.add)

            return do_group

        def ln_transpose_project(x_dram, wname, mode, copy_eng):
            """Returns do_group(g): LN -> z -> zT -> projection for token tiles
            4g..4g+3. mode: 'fm' (feature-major out into QT/KT) or 'tm'
            (token-major out into V65). copy_eng: 'act' or 'dve' for the
            PSUM->SBUF transpose copyback."""
            zt = ztp.tile([128, NIC, S], TDT, tag="zt")
            w_sb = w_sbs[wname]

            def proj_group(n):
                # token-range n*512:(n+1)*512 of zT is complete
                if mode == "fm":
                    dst = QT if wname == "q" else KT
                    b_sb = bq_sb if wname == "q" else bk_sb
                    for m in range(2):
                        ps = pp_mm.tile([128, 512], F32, tag="mm")
                        for ic in range(NIC):
                            nc.tensor.matmul(
                                ps[:, :],
                                lhsT=w_sb[:, ic, m * 128:(m + 1) * 128],
                                rhs=zt[:, ic, n * 512:(n + 1) * 512],
                                start=(ic == 0), stop=(ic == NIC - 1))
                        nc.vector.tensor_scalar(
                            out=dst[:, m, n * 512:(n + 1) * 512], in0=ps[:, :],
                            scalar1=b_sb[:, m:m + 1],
                            scalar2=None, op0=mybir.AluOpType.add)
                else:
                    for j in range(4 * n, 4 * n + 4):
                        ps = pp_mm.tile([128, HS], F32, tag="mm")
                        for ic in range(NIC):
                            nc.tensor.matmul(
                                ps[:, :],
                                lhsT=zt[:, ic, j * 128:(j + 1) * 128],
                                rhs=w_sb[:, ic, :],
                                start=(ic == 0), stop=(ic == NIC - 1))
                        for h in range(HPC):
                            nc.vector.tensor_tensor(
                                out=V65[:, j, h, 0:DH],
                                in0=ps[:, h * DH:(h + 1) * DH],
                                in1=bv_sb[:, h * DH:(h + 1) * DH],
                                op=mybir.AluOpType.add)

            def do_group(grp):
                for j in range(4 * grp, 4 * grp + 4):
                    x_t = xp.tile([128, D], XDT)
                    nc.sync.dma_start(out=x_t[:, :],
                                      in_=x_dram[j * 128:(j + 1) * 128, :])
                    if T["ablate"] in ("noln", "notrans"):
                        if T["ablate"] == "notrans":
                            nc.sync.dma_start(
                                out=zt[:, :, j * 128:(j + 1) * 128],
                                in_=x_dram[j * 128:(j + 1) * 128, :].rearrange(
                                    "p (c q) -> p c q", c=NIC))
                        else:
                            tp = pp_mm.tile([128, NIC, 128], TDT, tag="mm")
                            for ic in range(NIC):
                                nc.tensor.transpose(
                                    tp[:, ic, :], x_t[:, ic * 128:(ic + 1) * 128],
                                    identity[:, :])
                            nc.vector.tensor_copy(
                                zt[:, :, j * 128:(j + 1) * 128], tp[:, :, :])
                        continue
                    if T["stats"] == "bn":
                        st = statp.tile([128, 2, 6], F32, tag="st")
                        for g in range(2):
                            nc.vector.bn_stats(out=st[:, g, :],
                                               in_=x_t[:, g * 512:(g + 1) * 512])
                        mv = statp.tile([128, 2], F32, tag="mv")
                        nc.vector.bn_aggr(out=mv[:, :], in_=st[:, :, :])
                        mu_ap, var_ap, var_scale = mv[:, 0:1], mv[:, 1:2], 1.0
                        mu_scale = 1.0
                    else:
                        # sum and sumsq via accum_out side-outputs (cheaper on
                        # DVE than bn_stats: 4x/2x modes apply)
                        sums = statp.tile([128, 2], F32, tag="mv")
                        trash = trash_t
                        nc.vector.tensor_scalar(
                            out=trash[:, :], in0=x_t[:, :], scalar1=1.0,
                            scalar2=None, op0=mybir.AluOpType.mult,
                            accum_out=sums[:, 0:1])
                        nc.vector.tensor_tensor_reduce(
                            out=trash[:, :], in0=x_t[:, :], in1=x_t[:, :],
                            scale=1.0, scalar=0.0, op0=mybir.AluOpType.mult,
                            op1=mybir.AluOpType.add, accum_out=sums[:, 1:2])
                        # var = (sumsq - sum^2/D)/D ; mu = sum/D
                        t1 = statp.tile([128, 1], F32, tag="st")
                        nc.vector.tensor_tensor(
                            out=t1[:, :], in0=sums[:, 0:1], in1=sums[:, 0:1],
                            op=mybir.AluOpType.mult)
                        varD = statp.tile([128, 1], F32, tag="varD")
                        nc.vector.tensor_scalar(
                            out=varD[:, :], in0=t1[:, :], scalar1=-1.0 / D,
                            scalar2=sums[:, 1:2], op0=mybir.AluOpType.mult,
                            op1=mybir.AluOpType.add)
                        mu_ap, var_ap = sums[:, 0:1], varD[:, :]
                        var_scale, mu_scale = 1.0 / D, 1.0 / D
                    # rstd = exp(-0.5*ln(var+eps)) — Ln+Exp share one table set
                    lnv = statp.tile([128, 1], F32, tag="lnv")
                    nc.scalar.activation(lnv[:, :], var_ap,
                                         mybir.ActivationFunctionType.Ln,
                                         bias=eps_t[:, :], scale=var_scale)
                    rstd = statp.tile([128, 1], F32, tag="rstd")
                    nc.scalar.activation(rstd[:, :], lnv[:, :],
                                         mybir.ActivationFunctionType.Exp,
                                         scale=-0.5)
                    nmur = statp.tile([128, 1], F32, tag="nmur")
                    nc.vector.tensor_scalar(
                        out=nmur[:, :], in0=mu_ap, scalar1=rstd[:, :],
                        scalar2=-mu_scale, op0=mybir.AluOpType.mult,
                        op1=mybir.AluOpType.mult)
                    z = zp.tile([128, D], TDT)
                    nc.gpsimd.tensor_scalar(
                        out=z[:, :], in0=x_t[:, :], scalar1=rstd[:, :],
                        scalar2=nmur[:, :], op0=mybir.AluOpType.mult,
                        op1=mybir.AluOpType.add)
                    if TRANS_ENG == "dma":
                        # xbar transpose engine: SBUF->SBUF, no PE/ACT/DVE cost
                        for ic in range(NIC):
                            nc.sync.dma_start_transpose(
                                out=zt[:, ic, j * 128:(j + 1) * 128],
                                in_=z[:, ic * 128:(ic + 1) * 128])
                    else:
                        tp = pp_mm.tile([128, NIC, 128], TDT, tag="mm")
                        for ic in range(NIC):
                            nc.tensor.transpose(tp[:, ic, :],
                                                z[:, ic * 128:(ic + 1) * 128],
                                                identity[:, :])
                        if copy_eng == "act":
                            # Copy is present in every ACT table set (no reload)
                            nc.scalar.activation(zt[:, :, j * 128:(j + 1) * 128],
                                                 tp[:, :, :],
                                                 mybir.ActivationFunctionType.Copy)
                        else:
                            nc.vector.tensor_copy(zt[:, :, j * 128:(j + 1) * 128],
                                                  tp[:, :, :])
                proj_group(grp)

            return do_group

        # k first (full K needed by every S^T tile), then q and v interleaved
        # per 4-tile group: the attention exp stream starts as soon as QT's
        # first quarter exists, and AV(kt) streams behind V65[kt] production.
        if T["lnmode"] == "fold":
            sk = ln_fold_project(xk, xkT, "k", "fm")
            sq = ln_fold_project(xq, xqT, "q", "fm")
            sv = ln_fold_project(xv, xvT, "v", "tm")
        else:
            sk = ln_transpose_project(xk, "k", "fm", T["copy_k"])
            sq = ln_transpose_project(xq, "q", "fm", T["copy_q"])
            sv = ln_transpose_project(xv, "v", "tm", T["copy_v"])
        for g in range(4):
            sk(g)
        for g in range(4):
            if T["v_first"]:
                sv(g)
                sq(g)
            else:
                sq(g)
                sv(g)

        # --- attention (qb outer so the output projection can stream) ---
        QBW = T.get("qbw", QB)     # attention q-block width (512 or 1024)
        PAIR = 1024 // QBW         # k-tiles paired per PSUM tile/exp
        for qb in range(S // QBW):
            for h in range(HPC):
                hc, ho = h // 2, 64 * (h % 2)
                p_tiles = []
                for kt2 in range(NT // PAIR):
                    if T["ablate"] == "noexp":
                        p = pb.tile([128, PAIR, QBW], MDT)
                        nc.vector.memset(p[:, 0:1, 0:1], 0.0)
                        p_tiles.append(p)
                        continue
                    # PAIR k-tiles share one 2-bank PSUM tile so a single exp
                    # covers 1024 columns (amortizes ACT per-op overhead)
                    st_ps = pp_st.tile([128, PAIR, QBW], F32)
                    for i in range(PAIR):
                        kt = kt2 * PAIR + i
                        nc.tensor.matmul(
                            st_ps[:, i, :],
                            lhsT=KT[ho:ho + DH, hc, kt * 128:(kt + 1) * 128],
                            rhs=QT[ho:ho + DH, hc, qb * QBW:(qb + 1) * QBW],
                            start=True, stop=True)
                    p = pb.tile([128, PAIR, QBW], MDT)
                    nc.scalar.activation(p[:, :, :], st_ps[:, :, :],
                                         mybir.ActivationFunctionType.Exp,
                                         scale=float(SCALE))
                    p_tiles.append(p)
                if T["ablate"] == "noav":
                    continue
                av = pp_av.tile([DH + 1, QBW], F32)
                for kt in range(NT):
                    nc.tensor.matmul(
                        av[:, :],
                        lhsT=V65[:, kt, h, :],
                        rhs=p_tiles[kt // PAIR][:, kt % PAIR, :],
                        start=(kt == 0), stop=(kt == NT - 1))
                # 1/denom = exp(-ln(denom)) on ACT: avoids the 1-lane DVE
                # iterative divide (~3.3us per row) and stays in the one
                # resident Exp/Ln table set.
                lnd = recp.tile([1, QBW], F32, tag="lnd")
                nc.scalar.activation(lnd[:, :], av[DH:DH + 1, :],
                                     mybir.ActivationFunctionType.Ln)
                rec = recp.tile([1, QBW], F32, tag="rec")
                nc.scalar.activation(rec[:, :], lnd[:, :],
                                     mybir.ActivationFunctionType.Exp,
                                     scale=-1.0)
                recb = recp.tile([DH, QBW], F32, tag="recb")
                nc.gpsimd.partition_broadcast(recb[:, :], rec[:, :])
                nc.vector.tensor_tensor(
                    out=CT[ho:ho + DH, hc, qb * QBW:(qb + 1) * QBW],
                    in0=av[0:DH, :], in1=recb[:, :],
                    op=mybir.AluOpType.mult)

            # output projection for this qb's token tiles (all heads done)
            for j in range(qb * QBW // 128, (qb + 1) * QBW // 128):
                ys = yp.tile([128, D], F32)
                for n in range(2):
                    ps = pp_mm.tile([128, 512], F32, tag="mm")
                    for cc in range(2):
                        nc.tensor.matmul(
                            ps[:, :],
                            lhsT=CT[:, cc, j * 128:(j + 1) * 128],
                            rhs=fo_sb[:, cc, n * 512:(n + 1) * 512],
                            start=(cc == 0), stop=(cc == 1))
                    nc.vector.tensor_copy(ys[:, n * 512:(n + 1) * 512], ps[:, :])
                nc.sync.dma_start(out=y[j * 128:(j + 1) * 128, :], in_=ys[:, :])

    nc.compile()
    return nc


_NC_CACHE = {}


def _get_nc():
    key = (XDT, TDT, WDT, MDT)
    if key not in _NC_CACHE:
        _NC_CACHE[key] = build_nc()
    return _NC_CACHE[key]


def make_in_maps(q, k, v, ln_g, ln_b, wq_w, wq_b, wk_w, wk_b, wv_w, wv_b, fo_w, fo_b):
    """Host-side shard prep. Folds ln_g/ln_b into projection weights/biases."""
    xnp = _NPDT[XDT]
    wnp = _NPDT[WDT]
    g64 = ln_g.astype(np.float64)
    b64 = ln_b.astype(np.float64)
    in_maps = []
    for c in range(N_CORES):
        b = c // 4
        sl = slice((c % 4) * HS, (c % 4 + 1) * HS)
        tnp = _NPDT[TDT]
        m = {
            "xq": np.ascontiguousarray(q[b]).astype(xnp),
            "xk": np.ascontiguousarray(k[b]).astype(xnp),
            "xv": np.ascontiguousarray(v[b]).astype(xnp),
            "xqT": np.ascontiguousarray(q[b].T).astype(tnp),
            "xkT": np.ascontiguousarray(k[b].T).astype(tnp),
            "xvT": np.ascontiguousarray(v[b].T).astype(tnp),
        }
        for nm, w, bias in (("q", wq_w, wq_b), ("k", wk_w, wk_b), ("v", wv_w, wv_b)):
            ws = w[sl].astype(np.float64)          # [256, 1024]
            wg = ws * g64[None, :]                 # fold gamma
            cb = (ws @ b64 + bias[sl].astype(np.float64)).astype(np.float32)
            m["w" + nm] = np.ascontiguousarray(wg.T).astype(wnp)  # [1024, 256]
            m["a1" + nm] = wg.sum(axis=1).astype(np.float32).reshape(1, HS).astype(wnp)
            if nm == "v":
                m["bv"] = cb.reshape(1, HS)
            else:
                m["b" + nm] = np.ascontiguousarray(cb.reshape(2, 128).T)  # [128, 2]
        m["fo"] = np.ascontiguousarray(fo_w[:, sl].T).astype(wnp)  # [256, 1024]
        in_maps.append(m)
    return in_maps


def run_on_device(in_maps, trace=False):
    nc = _get_nc()
    return bass_utils.run_bass_kernel_spmd(
        nc, in_maps, core_ids=list(range(N_CORES)), trace=trace)


def assemble(res, fo_b):
    """Gather-reduce the row-parallel partials and add the output bias."""
    fo_b64 = np.asarray(fo_b, np.float64)
    out = np.empty((B, S, D), np.float32)
    for b in range(B):
        acc = np.zeros((S, D), np.float64)
        for c in range(b * 4, b * 4 + 4):
            acc += res.results[c]["y"].astype(np.float64)
        out[b] = (acc + fo_b64[None, :]).astype(np.float32)
    return out


def kernel(q, k, v, ln_g, ln_b, wq_w, wq_b, wk_w, wk_b, wv_w, wv_b, fo_w, fo_b):
    q = np.asarray(q, np.float32)
    k = np.asarray(k, np.float32)
    v = np.asarray(v, np.float32)
    in_maps = make_in_maps(q, k, v, np.asarray(ln_g, np.float32),
                           np.asarray(ln_b, np.float32),
                           np.asarray(wq_w, np.float32), np.asarray(wq_b, np.float32),
                           np.asarray(wk_w, np.float32), np.asarray(wk_b, np.float32),
                           np.asarray(wv_w, np.float32), np.asarray(wv_b, np.float32),
                           np.asarray(fo_w, np.float32), np.asarray(fo_b, np.float32))
    res = run_on_device(in_maps)
    return assemble(res, fo_b)
